# revision 48
# baseline (speedup 1.0000x reference)
"""Trainium2 Bass kernel for nn_ArrivalTime (sparse attention over 24 timeslots).

Math refactoring (exact, up to fp reassociation):
  query = [user_pref[user], timeslot[hour]] has only 64 distinct user rows and
  24 distinct time rows, so
    scores[n,h,t] = US[b(n), h, t] + TS[hour[n], h, t]
  with tiny host-precomputed tables; US is folded into a per-batch-row A-table
  (the stream carries a constant ones-row), so no activation bias is needed.
  Masking adds -1e9 where hour_mask==1.  Softmax per head over t (24).
  Output: out[n,:] = attn[n,:] @ vproj + bu, vproj[(h,t),d] = v[h,t,:]@Wu[d,h*HD:]^T.

Device pipeline (per core, transposed layout: tokens on the free dim), one
iteration per batch row (512 tokens), matmul operands bf16:
  PE : ps_s = table_b^T @ stream    (one-hot hour + mask + ones rows, K=49;
                                     consecutive rows are PAIRED into disjoint
                                     PE row groups via tile_position and run
                                     concurrently, ~2x the A throughput)
  ACT: p = exp(ps_s)                (row 96 = exp(0) = 1 -> carries bu)
  PE : ps_z = seg2^T @ p            (per-head sums replicated, [96,S] psum x2)
  ACT: lnz = ln(ps_z); r = exp(-lnz)  (PER-ROW: short dependency chains keep
                                     more iterations in flight; the kernel is
                                     latency-bound, not throughput-bound)
  DVE: p[:96] *= r slice            (bf16 all-SBUF -> 2x mode)
  PE : ps_o = vproj_ext^T @ p       (two halves into one 2-bank f32 psum, x2)
  DVE: ot = ps_o                    (single [128,2S] f32->bf16 cast; the two
                                     tail rows' casts run on the by-then-idle
                                     ACT in parallel with the DVE)
  SP : one bf16 output DMA per iteration.
A 7-matmul warm-up burst on memset scratch during the input-DMA flight window
releases the PE HAM clock throttle (K=4/8 -> 8/8) as real work begins.  All
DMAs are issued from the SP ring (16 DMA engines; the ACT hwdge ring maps
to a single engine and is ~5x slower for multi-descriptor transfers); vpseg
is split into two jobs for two-engine parallelism.  GpSimd is never used: it
cannot touch PSUM and its SBUF traffic slows every other engine ~20%.  m2
lags 2 iterations; standalone wait_ge synchronization throughout, with
same-engine write->read self-waits (the ACT ack path is pipelined and the
race detector treats unsynchronized same-engine RAW/WAR as hazards).

Sharding: data-parallel over batch, 8 batch rows (= 8 x 512 tokens) per core.
"""

import os
import numpy as np

B, S, D, H, HD, T = 64, 512, 256, 4, 64, 24
NCORES = 8
BPC = B // NCORES  # batch rows per core
HT = H * T  # 96
K1 = 2 * T + 1  # 49 stream rows: one-hot hour + mask + ones
MASK_NEG = -1.0e9
TW = HT + 1  # 97: table columns / p partitions

# vpseg bf16 constant tensor [97, VW]: vproj_ext then seg2
C_VP = 0
C_SEG2 = C_VP + D
VW = C_SEG2 + HT


def _host_tables(timeslot_embedded, user, hour, hour_mask, user_pref,
                 Wq, bq, Wk, bk, Wv, bv, Wu, bu):
    import ml_dtypes
    f32 = np.float32
    bf16 = ml_dtypes.bfloat16
    ts_e = np.asarray(timeslot_embedded, f32)          # [T, D]
    user = np.asarray(user).astype(np.int64)           # [B]
    hour = np.asarray(hour).astype(np.int64)           # [B, S]
    hour_mask = np.asarray(hour_mask)                  # [B, S, T]
    Wq = np.asarray(Wq, f32); bq = np.asarray(bq, f32)
    Wk = np.asarray(Wk, f32); bk = np.asarray(bk, f32)
    Wv = np.asarray(Wv, f32); bv = np.asarray(bv, f32)
    Wu = np.asarray(Wu, f32); bu = np.asarray(bu, f32)

    Wq_u, Wq_t = Wq[:, :, :D], Wq[:, :, D:]
    k_ = np.einsum('td,hkd->htk', ts_e, Wk) + bk[:, None, :]   # [H,T,HD]
    v_ = np.einsum('td,hkd->htk', ts_e, Wv) + bv[:, None, :]
    time_q = np.einsum('td,hkd->thk', ts_e, Wq_t)              # [T,H,HD]
    upref = np.asarray(user_pref, f32)[user]                   # [B,D]
    user_q = np.einsum('bd,hkd->bhk', upref, Wq_u) + bq[None]  # [B,H,HD]
    scale = f32(1.0 / np.sqrt(HD))
    TS = (np.einsum('thk,hsk->ths', time_q, k_) * scale).reshape(T, HT)
    US = (np.einsum('bhk,hsk->bhs', user_q, k_) * scale).reshape(B, HT)
    vproj = np.einsum('htk,dhk->htd', v_, Wu.reshape(D, H, HD)).reshape(HT, D)

    # per-batch-row tables [K1, TW]: rows 0..23 TS, rows 24..47 mask additive,
    # row 48 = US_b (ones-row of the stream); col 96 = 0 everywhere.
    # Rows are PAIR-STACKED on the partition axis (row 2m at partitions 0:49,
    # row 2m+1 at 64:64+49) so the two A-matmuls of a pair run concurrently
    # in disjoint PE row groups via tile_position.
    maskrows = np.tile(np.eye(T, dtype=f32), (1, H)) * f32(MASK_NEG)
    tabs_cores = []
    for c in range(NCORES):
        tc = np.zeros((128, (BPC // 2) * TW), f32)
        for j in range(BPC):
            b = c * BPC + j
            r0 = 64 * (j % 2)
            co = (j // 2) * TW
            tc[r0:r0 + T, co:co + HT] = TS
            tc[r0 + T:r0 + 2 * T, co:co + HT] = maskrows
            tc[r0 + 2 * T, co:co + HT] = US[b]
        tabs_cores.append(tc.astype(bf16))

    seg2 = np.kron(np.eye(H, dtype=f32), np.ones((T, T), f32))  # [HT, HT]
    vs = np.zeros((TW, VW), f32)
    vs[:HT, C_VP:C_VP + D] = vproj
    vs[HT, C_VP:C_VP + D] = bu
    vs[:HT, C_SEG2:C_SEG2 + HT] = seg2
    vpseg_bf = vs.astype(bf16)

    # per-core streams, pair-stacked like the tables: [BPC//2, 128, S]
    eyeT = np.eye(T, dtype=f32)
    streams = []
    for c in range(NCORES):
        hb = hour[c * BPC:(c + 1) * BPC]                       # [BPC, S]
        mb = hour_mask[c * BPC:(c + 1) * BPC]                  # [BPC, S, T]
        st = np.zeros((BPC // 2, 128, S), f32)
        for j in range(BPC):
            r0 = 64 * (j % 2)
            st[j // 2, r0:r0 + T, :] = eyeT[hb[j]].T
            st[j // 2, r0 + T:r0 + 2 * T, :] = mb[j].astype(f32).T
            st[j // 2, r0 + 2 * T, :] = 1.0
        streams.append(st.astype(bf16))
    return tabs_cores, vpseg_bf, streams


def _build_program():
    import concourse.bass as bass
    import concourse.mybir as mybir
    from contextlib import ExitStack

    class _NoBarrierBlock(bass.BassBlock):
        # The stock Block.__exit__ emits per-engine drains plus a full
        # all-engine semaphore barrier whose wakeup costs ~6-8us of tail.
        # Output completion is already guaranteed by the explicit ot_sem
        # waits on the sync engine.
        def __exit__(self, exc_type, exc_val, exc_tb):
            if exc_type is None:
                for engine, last_body in self.last_body.items():
                    with self.bass.body(last_body, parent=self.bass.cur_bb,
                                        allow_existing_parent=True):
                        engine.br(self.end_bb)
                self.bass.switch_bb(self.end_bb)

    f32 = mybir.dt.float32
    bf16 = mybir.dt.bfloat16
    nc = bass.Bass("TRN2")
    stream_d = nc.declare_dram_parameter("stream", [BPC // 2, 128, S], bf16,
                                         isOutput=False)
    tabs_d = nc.declare_dram_parameter("tabs", [128, (BPC // 2) * TW], bf16,
                                       isOutput=False)
    vpseg_d = nc.declare_dram_parameter("vpseg", [TW, VW], bf16,
                                        isOutput=False)
    out_d = nc.declare_dram_parameter("out", [BPC, D, S], bf16, isOutput=True)

    Exp = mybir.ActivationFunctionType.Exp
    Ln = mybir.ActivationFunctionType.Ln
    LAG = 2  # m2 lag

    with ExitStack() as ctx:
        ec = ctx.enter_context
        tabs_sb = ec(nc.sbuf_tensor("tabs_sb", [128, (BPC // 2) * TW], bf16))
        vpseg_sb = ec(nc.sbuf_tensor("vpseg_sb", [TW, VW], bf16))
        sts = [ec(nc.sbuf_tensor(f"st{j}", [128, S], bf16))
               for j in range(BPC // 2)]
        pw = [ec(nc.sbuf_tensor(f"p{j}", [TW, 2 * S], bf16))
              for j in range(3)]
        lnz_sb = ec(nc.sbuf_tensor("lnz_sb", [HT, 2 * S], f32))
        r_sb = ec(nc.sbuf_tensor("r_sb", [HT, 2 * S], bf16))
        ots = [ec(nc.sbuf_tensor(f"ot{j}", [128, 2 * S], bf16))
               for j in range(3)]
        warm_sb = ec(nc.sbuf_tensor("warm_sb", [128, S], bf16))
        ps_sw = ec(nc.psum_tensor("ps_sw", [TW, 2 * S], f32))
        zws = [ec(nc.psum_tensor(f"zw{j}", [HT, S], f32)) for j in range(2)]
        ps_os = [ec(nc.psum_tensor(f"ps_o{j}", [128, 2 * S], f32))
                 for j in range(2)]
        warm_sem = ec(nc.semaphore("warm_sem"))
        c_sem = ec(nc.semaphore("c_sem"))     # tabs DMA
        v_sem = ec(nc.semaphore("v_sem"))     # vpseg DMA
        st_sems = [ec(nc.semaphore(f"st_sem{j}")) for j in range(BPC // 2)]
        pe_sem = ec(nc.semaphore("pe_sem"))
        act_sem = ec(nc.semaphore("act_sem"))
        dve_sem = ec(nc.semaphore("dve_sem"))
        ot_sems = [ec(nc.semaphore(f"ot_sem{j}")) for j in range(BPC)]
        nc.check_frozen()
        block = ec(_NoBarrierBlock(nc, f"block_{nc.next_id()}"))
        nc.cur_block = block

        vproj = vpseg_sb[:, C_VP:C_VP + D]
        seg2 = vpseg_sb[:, C_SEG2:C_SEG2 + HT]

        def tab(i):
            r0 = 64 * (i % 2)
            return tabs_sb[r0:r0 + K1, (i // 2) * TW:(i // 2 + 1) * TW]

        pe_tick = {}
        act_tick = {}
        dve_tick = {}
        _cnt = {'pe': 0, 'act': 0, 'dve': 0}

        def _rec(tickmap, cnt_key, key):
            _cnt[cnt_key] += 1
            tickmap[key] = _cnt[cnt_key]

        # ACT emission order: exp runs one row ahead; the pair ln/expneg is
        # emitted after exp_{2m+2} so it never blocks the next exp.
        # rows 0,1 and BPC-2,BPC-1 use per-row ln/expneg ([96,S] halves) to
        # shorten the pipeline-fill and end-of-kernel chains; middle pairs
        # (2,3) and (4,5) batch ln/expneg on the [96,2S] wide psum.
        # fully per-row ln/expneg: shorter dependency chains per row keep
        # more iterations in flight (latency-bound regime)
        act_order = [('ep', 0)]
        for i in range(BPC):
            act_order.append(('lnr', i))
            act_order.append(('expnegr', i))
            if i % 2 == 1 and (i + 1) // 2 + 1 < BPC // 2 + 1 and \
                    i + 1 < BPC:
                act_order.append(('ep', (i + 1) // 2))
        act_order.append(('acp', BPC - 2))
        act_order.append(('acpB', BPC - 1))

        def rkey(i):                  # tick that makes row i's r slice ready
            return ('expnegr', i)

        ZW_FREE = {j: ('lnr', j - 2) for j in range(2, BPC)}
        for key in act_order:
            _rec(act_tick, 'act', key)

        # DVE emission order: ocopy first (its data arrives earlier), then mul
        dve_order = []
        for i in range(BPC + LAG):
            if i >= LAG:
                k = i - LAG
                if k == BPC - 1:
                    dve_order.append(('ocopyA', k))
                elif k != BPC - 2:  # row BPC-2 and the B half of BPC-1
                    dve_order.append(('ocopy', k))  # are copied on ACT
            if i < BPC:
                dve_order.append(('mul', i))
        for key in dve_order:
            _rec(dve_tick, 'dve', key)

        @block.tensor
        def _(tensor):
            def mm(key, out, lhsT, rhs, tile_position=None):
                tensor.matmul(out, lhsT, rhs, start=True, stop=True,
                              tile_position=tile_position).then_inc(pe_sem, 1)
                _rec(pe_tick, 'pe', key)

            # HAM warm-up: 7 x 512-col matmuls on memset scratch during the
            # input-DMA flight window release the PE clock throttle
            # (K=4/8 -> 8/8) right as the real work begins.  Results land in
            # ps_os[0] which m2_0 later overwrites with start=True.
            def warm_mm(n=S):
                tensor.matmul(ps_os[0][:, 0:n], warm_sb[:, 0:128],
                              warm_sb[:, 0:n], start=True, stop=True)

            def a_pair(m):
                # the two A-matmuls of pair m run concurrently in disjoint
                # PE row groups (rows 0-48 and 64-112), into the two halves
                # (= two banks) of the wide score psum
                mm(('A', 2 * m), ps_sw[:, 0:S], tab(2 * m),
                   sts[m][0:K1, :], tile_position=(0, 0))
                mm(('A', 2 * m + 1), ps_sw[:, S:2 * S], tab(2 * m + 1),
                   sts[m][64:64 + K1, :], tile_position=(64, 0))

            tensor.wait_ge(warm_sem, 1)
            for _ in range(7):
                warm_mm()
            tensor.wait_ge(c_sem, 16)
            tensor.wait_ge(st_sems[0], 16)
            a_pair(0)
            for j in range(BPC + LAG):
                if j % 2 == 1 and (j + 1) // 2 < BPC // 2:  # A pair (j+1)//2
                    m = (j + 1) // 2
                    tensor.wait_ge(st_sems[m], 16)
                    # both ps_sw banks freed by the previous pair's exp
                    tensor.wait_ge(act_sem, act_tick[('ep', m - 1)])
                    a_pair(m)
                if j < BPC:                     # hs_j
                    tensor.wait_ge(act_sem, act_tick[('ep', j // 2)])
                    if j == 0:
                        tensor.wait_ge(v_sem, 32)
                    if j in ZW_FREE:            # previous reader freed zw half
                        tensor.wait_ge(act_sem, act_tick[ZW_FREE[j]])
                    pws = pw[(j // 2) % 3][:, (j % 2) * S:(j % 2 + 1) * S]
                    mm(('hs', j), zws[j % 2][:], seg2, pws)
                if 0 <= j - LAG < BPC:          # m2_{j-LAG}
                    i = j - LAG
                    tensor.wait_ge(dve_sem, dve_tick[('mul', i)])
                    if i == 0:
                        tensor.wait_ge(v_sem, 32)
                    if i >= 2:                  # copy of row i-2 freed ps_o
                        tensor.wait_ge(dve_sem, dve_tick[('ocopy', i - 2)])
                    pwi = pw[(i // 2) % 3][:, (i % 2) * S:(i % 2 + 1) * S]
                    mm(('m2a', i), ps_os[i % 2][:, 0:S],
                       vproj[:, 0:128], pwi)
                    mm(('m2b', i), ps_os[i % 2][:, S:2 * S],
                       vproj[:, 128:256], pwi)


        @block.scalar
        def _(scalar):
            # preload the Exp/Ln PWP tables during the DMA flight window
            # (input is the preamble-initialized const-1.0 AP so CoreSim's
            # uninitialized-read check stays clean; scratch out into ot0)
            cap = nc.const_aps.aps[(f32, 1.0)]
            scalar.activation(lnz_sb[:4, 0:1], cap[0:4], Exp)
            scalar.activation(lnz_sb[:4, 1:2], cap[0:4], Ln)
            for key in act_order:
                kind, i = key
                if kind == 'ep':            # wide exp for pair i
                    scalar.wait_ge(pe_sem, pe_tick[('A', 2 * i + 1)])
                    scalar.activation(pw[i % 3][:], ps_sw[:],
                                      Exp).then_inc(act_sem, 1)
                elif kind == 'lnr':             # per-row ln, row i
                    sl = (i % 2) * S
                    scalar.wait_ge(pe_sem, pe_tick[('hs', i)])
                    if i >= 2:  # lnz-slice WAR: expnegr_{i-2} reads drained
                        scalar.wait_ge(act_sem, act_tick[('expnegr', i - 2)])
                    scalar.activation(lnz_sb[:, sl:sl + S],
                                      zws[i % 2][:],
                                      Ln).then_inc(act_sem, 1)
                elif kind == 'expnegr':     # per-row expneg, row i
                    sl = (i % 2) * S
                    if i >= 2:                  # mul_{i-2} freed r_sb slice
                        scalar.wait_ge(dve_sem, dve_tick[('mul', i - 2)])
                    scalar.wait_ge(act_sem, act_tick[('lnr', i)])
                    scalar.activation(r_sb[:, sl:sl + S],
                                      lnz_sb[:, sl:sl + S], Exp,
                                      scale=-1.0).then_inc(act_sem, 1)
                elif kind == 'acp':             # tail output copy, row i
                    scalar.wait_ge(pe_sem, pe_tick[('m2b', i)])
                    scalar.wait_ge(ot_sems[i - 3], 16)
                    scalar.copy(ots[i % 3][:],
                                ps_os[i % 2][:]).then_inc(act_sem, 1)
                else:  # acpB: tail output copy, row i's B half
                    scalar.wait_ge(pe_sem, pe_tick[('m2b', i)])
                    scalar.wait_ge(ot_sems[i - 3], 16)
                    scalar.copy(ots[i % 3][:, S:2 * S],
                                ps_os[i % 2][:, S:2 * S]).then_inc(act_sem, 1)

        @block.vector
        def _(vector):
            vector.memset(warm_sb[:], 0.0).then_inc(warm_sem, 1)
            for key in dve_order:
                kind, i = key
                if kind == 'mul':
                    vector.wait_ge(act_sem, act_tick[rkey(i)])
                    sl = (i % 2) * S
                    pwi = pw[(i // 2) % 3][:HT, sl:sl + S]
                    vector.tensor_mul(pwi, pwi,
                                      r_sb[:, sl:sl + S]).then_inc(dve_sem, 1)
                elif kind == 'ocopy':
                    vector.wait_ge(pe_sem, pe_tick[('m2b', i)])
                    if i >= 3:
                        vector.wait_ge(ot_sems[i - 3], 16)
                    vector.tensor_copy(ots[i % 3][:],
                                       ps_os[i % 2][:]).then_inc(dve_sem, 1)
                elif kind == 'ocopyA':
                    vector.wait_ge(pe_sem, pe_tick[('m2a', i)])
                    if i >= 3:
                        vector.wait_ge(ot_sems[i - 3], 16)
                    vector.tensor_copy(ots[i % 3][:, 0:S],
                                       ps_os[i % 2][:, 0:S]).then_inc(
                                           dve_sem, 1)


        @block.sync
        def _(sync):
            sync.dma_start(sts[0][:], stream_d[0]).then_inc(st_sems[0], 16)
            sync.dma_start(tabs_sb[:], tabs_d[:]).then_inc(c_sem, 16)
            sync.dma_start(vpseg_sb[0:49, :],
                           vpseg_d[0:49, :]).then_inc(v_sem, 16)
            sync.dma_start(vpseg_sb[49:TW, :],
                           vpseg_d[49:TW, :]).then_inc(v_sem, 16)
            sync.dma_start(sts[1][:], stream_d[1]).then_inc(st_sems[1], 16)
            for i in range(2, BPC // 2):
                sync.dma_start(sts[i][:], stream_d[i]).then_inc(st_sems[i], 16)
            for k in range(BPC - 1):
                if k == BPC - 2:
                    sync.wait_ge(act_sem, act_tick[('acp', k)])
                else:
                    sync.wait_ge(dve_sem, dve_tick[('ocopy', k)])
                dest = out_d[k, :, :].rearrange("(h p) s -> p h s", h=2)
                src = ots[k % 3][:, :].rearrange("p (h s) -> p h s", h=2)
                sync.dma_start(dest, src).then_inc(ot_sems[k], 16)
            k = BPC - 1
            sync.wait_ge(dve_sem, dve_tick[('ocopyA', k)])
            sync.dma_start(out_d[k, 0:128, :],
                           ots[k % 3][:, 0:S]).then_inc(ot_sems[k], 16)
            sync.wait_ge(act_sem, act_tick[('acpB', k)])
            sync.dma_start(out_d[k, 128:256, :],
                           ots[k % 3][:, S:2 * S]).then_inc(ot_sems[k], 16)
            for k in range(BPC - 1):
                sync.wait_ge(ot_sems[k], 16)
            sync.wait_ge(ot_sems[BPC - 1], 32)

    return nc


def _run(inputs, trace=False):
    import sys
    if "/opt/trn_rl_repo" not in sys.path:
        sys.path.insert(0, "/opt/trn_rl_repo")
    from concourse.bass_utils import run_bass_kernel_spmd

    tabs_cores, vpseg_bf, streams = _host_tables(**inputs)
    nc = _build_program()
    in_maps = [
        {"stream": streams[c], "tabs": tabs_cores[c], "vpseg": vpseg_bf}
        for c in range(NCORES)
    ]
    res = run_bass_kernel_spmd(nc, in_maps, core_ids=list(range(NCORES)),
                               trace=trace)
    out_full = np.empty((B, S, D), np.float32)
    for c in range(NCORES):
        oc = res.results[c]["out"]  # [BPC, D, S] bf16
        out_full[c * BPC:(c + 1) * BPC] = \
            oc.astype(np.float32).transpose(0, 2, 1)
    return out_full, res


def kernel(**inputs):
    trace = bool(int(os.environ.get("BASS_KERNEL_TRACE", "0")))
    out, _ = _run(inputs, trace=trace)
    return out


def kernel_profiled(**inputs):
    out, res = _run(inputs, trace=True)
    return out, res


# revision 49
# speedup vs baseline: 1.3661x; 1.3661x over previous
"""Trainium2 Bass kernel for nn_ArrivalTime (sparse attention over 24 timeslots).

Math refactoring (exact, up to fp reassociation):
  query = [user_pref[user], timeslot[hour]] has only 64 distinct user rows and
  24 distinct time rows, so
    scores[n,h,t] = US[b(n), h, t] + TS[hour[n], h, t]
  with tiny host-precomputed tables; US is folded into a per-batch-row A-table
  (the stream carries a constant ones-row), so no activation bias is needed.
  Masking adds -1e9 where hour_mask==1.  Softmax per head over t (24).
  Output: out[n,:] = attn[n,:] @ vproj + bu, vproj[(h,t),d] = v[h,t,:]@Wu[d,h*HD:]^T.

Device pipeline (per core, transposed layout: tokens on the free dim), one
iteration per batch row (512 tokens), matmul operands bf16:
  PE : ps_s = table_b^T @ stream    (one-hot hour + mask + ones rows, K=49;
                                     consecutive rows are PAIRED into disjoint
                                     PE row groups via tile_position and run
                                     concurrently, ~2x the A throughput)
  ACT: p = exp(ps_s)                (row 96 = exp(0) = 1 -> carries bu)
  PE : ps_z = seg2^T @ p            (per-head sums replicated, [96,S] psum x2)
  ACT: lnz = ln(ps_z); r = exp(-lnz)  (PER-ROW: short dependency chains keep
                                     more iterations in flight; the kernel is
                                     latency-bound, not throughput-bound)
  DVE: p[:96] *= r slice            (bf16 all-SBUF -> 2x mode)
  PE : ps_o = vproj_ext^T @ p       (two halves into one 2-bank f32 psum, x2)
  DVE: ot = ps_o                    (single [128,2S] f32->bf16 cast; the two
                                     tail rows' casts run on the by-then-idle
                                     ACT in parallel with the DVE)
  SP : one bf16 output DMA per iteration.
A 7-matmul warm-up burst on memset scratch during the input-DMA flight window
releases the PE HAM clock throttle (K=4/8 -> 8/8) as real work begins.  All
DMAs are issued from the SP ring (16 DMA engines; the ACT hwdge ring maps
to a single engine and is ~5x slower for multi-descriptor transfers); vpseg
is split into two jobs for two-engine parallelism.  GpSimd is never used: it
cannot touch PSUM and its SBUF traffic slows every other engine ~20%.  m2
lags 2 iterations; standalone wait_ge synchronization throughout, with
same-engine write->read self-waits (the ACT ack path is pipelined and the
race detector treats unsynchronized same-engine RAW/WAR as hazards).

Sharding: data-parallel over batch, 8 batch rows (= 8 x 512 tokens) per core.
"""

import os
import numpy as np

B, S, D, H, HD, T = 64, 512, 256, 4, 64, 24
NCORES = 8
BPC = B // NCORES  # batch rows per core
HT = H * T  # 96
K1 = 2 * T + 1  # 49 stream rows: one-hot hour + mask + ones
MASK_NEG = -1.0e9
TW = HT + 1  # 97: table columns / p partitions

# vpseg bf16 constant tensor [97, VW]: vproj_ext then seg2
C_VP = 0
C_SEG2 = C_VP + D
VW = C_SEG2 + HT


def _host_tables(timeslot_embedded, user, hour, hour_mask, user_pref,
                 Wq, bq, Wk, bk, Wv, bv, Wu, bu):
    import ml_dtypes
    f32 = np.float32
    bf16 = ml_dtypes.bfloat16
    ts_e = np.asarray(timeslot_embedded, f32)          # [T, D]
    user = np.asarray(user).astype(np.int64)           # [B]
    hour = np.asarray(hour).astype(np.int64)           # [B, S]
    hour_mask = np.asarray(hour_mask)                  # [B, S, T]
    Wq = np.asarray(Wq, f32); bq = np.asarray(bq, f32)
    Wk = np.asarray(Wk, f32); bk = np.asarray(bk, f32)
    Wv = np.asarray(Wv, f32); bv = np.asarray(bv, f32)
    Wu = np.asarray(Wu, f32); bu = np.asarray(bu, f32)

    Wq_u, Wq_t = Wq[:, :, :D], Wq[:, :, D:]
    k_ = np.einsum('td,hkd->htk', ts_e, Wk) + bk[:, None, :]   # [H,T,HD]
    v_ = np.einsum('td,hkd->htk', ts_e, Wv) + bv[:, None, :]
    time_q = np.einsum('td,hkd->thk', ts_e, Wq_t)              # [T,H,HD]
    upref = np.asarray(user_pref, f32)[user]                   # [B,D]
    user_q = np.einsum('bd,hkd->bhk', upref, Wq_u) + bq[None]  # [B,H,HD]
    scale = f32(1.0 / np.sqrt(HD))
    TS = (np.einsum('thk,hsk->ths', time_q, k_) * scale).reshape(T, HT)
    US = (np.einsum('bhk,hsk->bhs', user_q, k_) * scale).reshape(B, HT)
    vproj = np.einsum('htk,dhk->htd', v_, Wu.reshape(D, H, HD)).reshape(HT, D)

    # per-batch-row tables [K1, TW]: rows 0..23 TS, rows 24..47 mask additive,
    # row 48 = US_b (ones-row of the stream); col 96 = 0 everywhere.
    # Rows are PAIR-STACKED on the partition axis (row 2m at partitions 0:49,
    # row 2m+1 at 64:64+49) so the two A-matmuls of a pair run concurrently
    # in disjoint PE row groups via tile_position.
    maskrows = np.tile(np.eye(T, dtype=f32), (1, H)) * f32(MASK_NEG)
    tabs_cores = []
    for c in range(NCORES):
        tc = np.zeros((128, (BPC // 2) * TW), f32)
        for j in range(BPC):
            b = c * BPC + j
            r0 = 64 * (j % 2)
            co = (j // 2) * TW
            tc[r0:r0 + T, co:co + HT] = TS
            tc[r0 + T:r0 + 2 * T, co:co + HT] = maskrows
            tc[r0 + 2 * T, co:co + HT] = US[b]
        tabs_cores.append(tc.astype(bf16))

    seg2 = np.kron(np.eye(H, dtype=f32), np.ones((T, T), f32))  # [HT, HT]
    vs = np.zeros((TW, VW), f32)
    vs[:HT, C_VP:C_VP + D] = vproj
    vs[HT, C_VP:C_VP + D] = bu
    vs[:HT, C_SEG2:C_SEG2 + HT] = seg2
    vpseg_bf = vs.astype(bf16)

    # per-core streams, pair-stacked like the tables: [BPC//2, 128, S]
    eyeT = np.eye(T, dtype=f32)
    streams = []
    for c in range(NCORES):
        hb = hour[c * BPC:(c + 1) * BPC]                       # [BPC, S]
        mb = hour_mask[c * BPC:(c + 1) * BPC]                  # [BPC, S, T]
        st = np.zeros((BPC // 2, 128, S), f32)
        for j in range(BPC):
            r0 = 64 * (j % 2)
            st[j // 2, r0:r0 + T, :] = eyeT[hb[j]].T
            st[j // 2, r0 + T:r0 + 2 * T, :] = mb[j].astype(f32).T
            st[j // 2, r0 + 2 * T, :] = 1.0
        streams.append(st.astype(bf16))
    return tabs_cores, vpseg_bf, streams


def _build_program():
    import concourse.bass as bass
    import concourse.mybir as mybir
    from contextlib import ExitStack

    class _NoBarrierBlock(bass.BassBlock):
        # The stock Block.__exit__ emits per-engine drains plus a full
        # all-engine semaphore barrier whose wakeup costs ~6-8us of tail.
        # Output completion is already guaranteed by the explicit ot_sem
        # waits on the sync engine.
        def __exit__(self, exc_type, exc_val, exc_tb):
            if exc_type is None:
                for engine, last_body in self.last_body.items():
                    with self.bass.body(last_body, parent=self.bass.cur_bb,
                                        allow_existing_parent=True):
                        engine.br(self.end_bb)
                self.bass.switch_bb(self.end_bb)

    f32 = mybir.dt.float32
    bf16 = mybir.dt.bfloat16
    nc = bass.Bass("TRN2")
    stream_d = nc.declare_dram_parameter("stream", [BPC // 2, 128, S], bf16,
                                         isOutput=False)
    tabs_d = nc.declare_dram_parameter("tabs", [128, (BPC // 2) * TW], bf16,
                                       isOutput=False)
    vpseg_d = nc.declare_dram_parameter("vpseg", [TW, VW], bf16,
                                        isOutput=False)
    out_d = nc.declare_dram_parameter("out", [BPC, D, S], bf16, isOutput=True)

    Exp = mybir.ActivationFunctionType.Exp
    Ln = mybir.ActivationFunctionType.Ln
    LAG = 2  # m2 lag

    with ExitStack() as ctx:
        ec = ctx.enter_context
        tabs_sb = ec(nc.sbuf_tensor("tabs_sb", [128, (BPC // 2) * TW], bf16))
        vpseg_sb = ec(nc.sbuf_tensor("vpseg_sb", [TW, VW], bf16))
        sts = [ec(nc.sbuf_tensor(f"st{j}", [128, S], bf16))
               for j in range(BPC // 2)]
        ps = [ec(nc.sbuf_tensor(f"p{j}", [TW, S], bf16)) for j in range(6)]
        lnz_sb = ec(nc.sbuf_tensor("lnz_sb", [HT, 2 * S], f32))
        r_sb = ec(nc.sbuf_tensor("r_sb", [HT, 2 * S], bf16))
        ots = [ec(nc.sbuf_tensor(f"ot{j}", [128, 2 * S], bf16))
               for j in range(3)]
        warm_sb = ec(nc.sbuf_tensor("warm_sb", [128, S], bf16))
        ps_ss = [ec(nc.psum_tensor(f"ps_s{j}", [TW, S], f32))
                 for j in range(2)]
        zws = [ec(nc.psum_tensor(f"zw{j}", [HT, S], f32)) for j in range(2)]
        ps_os = [ec(nc.psum_tensor(f"ps_o{j}", [128, 2 * S], f32))
                 for j in range(2)]
        warm_sem = ec(nc.semaphore("warm_sem"))
        c_sem = ec(nc.semaphore("c_sem"))     # tabs DMA
        v_sem = ec(nc.semaphore("v_sem"))     # vpseg DMA
        st_sems = [ec(nc.semaphore(f"st_sem{j}")) for j in range(BPC // 2)]
        pe_sem = ec(nc.semaphore("pe_sem"))
        act_sem = ec(nc.semaphore("act_sem"))
        dve_sem = ec(nc.semaphore("dve_sem"))
        ot_sems = [ec(nc.semaphore(f"ot_sem{j}")) for j in range(BPC)]
        nc.check_frozen()
        block = ec(_NoBarrierBlock(nc, f"block_{nc.next_id()}"))
        nc.cur_block = block

        vproj = vpseg_sb[:, C_VP:C_VP + D]
        seg2 = vpseg_sb[:, C_SEG2:C_SEG2 + HT]

        def tab(i):
            r0 = 64 * (i % 2)
            return tabs_sb[r0:r0 + K1, (i // 2) * TW:(i // 2 + 1) * TW]

        pe_tick = {}
        act_tick = {}
        dve_tick = {}
        _cnt = {'pe': 0, 'act': 0, 'dve': 0}

        def _rec(tickmap, cnt_key, key):
            _cnt[cnt_key] += 1
            tickmap[key] = _cnt[cnt_key]

        # ACT emission order: exp runs one row ahead; the pair ln/expneg is
        # emitted after exp_{2m+2} so it never blocks the next exp.
        # rows 0,1 and BPC-2,BPC-1 use per-row ln/expneg ([96,S] halves) to
        # shorten the pipeline-fill and end-of-kernel chains; middle pairs
        # (2,3) and (4,5) batch ln/expneg on the [96,2S] wide psum.
        # fully per-row ln/expneg: shorter dependency chains per row keep
        # more iterations in flight (latency-bound regime)
        act_order = [('exp', 0), ('exp', 1)]
        for i in range(BPC):
            act_order.append(('lnr', i))
            act_order.append(('expnegr', i))
            if i + 2 < BPC:
                act_order.append(('exp', i + 2))
        act_order.append(('acp', BPC - 2))
        act_order.append(('acpB', BPC - 1))

        def rkey(i):                  # tick that makes row i's r slice ready
            return ('expnegr', i)

        ZW_FREE = {j: ('lnr', j - 2) for j in range(2, BPC)}
        for key in act_order:
            _rec(act_tick, 'act', key)

        # DVE emission order: ocopy first (its data arrives earlier), then mul
        dve_order = []
        for i in range(BPC + LAG):
            if i >= LAG:
                k = i - LAG
                if k == BPC - 1:
                    dve_order.append(('ocopyA', k))
                elif k != BPC - 2:  # row BPC-2 and the B half of BPC-1
                    dve_order.append(('ocopy', k))  # are copied on ACT
            if i < BPC:
                dve_order.append(('mul', i))
        for key in dve_order:
            _rec(dve_tick, 'dve', key)

        @block.tensor
        def _(tensor):
            def mm(key, out, lhsT, rhs, tile_position=None):
                tensor.matmul(out, lhsT, rhs, start=True, stop=True,
                              tile_position=tile_position).then_inc(pe_sem, 1)
                _rec(pe_tick, 'pe', key)

            # HAM warm-up: 7 x 512-col matmuls on memset scratch during the
            # input-DMA flight window release the PE clock throttle
            # (K=4/8 -> 8/8) right as the real work begins.  Results land in
            # ps_os[0] which m2_0 later overwrites with start=True.
            def warm_mm(n=S):
                tensor.matmul(ps_os[0][:, 0:n], warm_sb[:, 0:128],
                              warm_sb[:, 0:n], start=True, stop=True)

            def a_pair(m):
                # the two A-matmuls of pair m run concurrently in disjoint
                # PE row groups (rows 0-48 and 64-112)
                mm(('A', 2 * m), ps_ss[0][:], tab(2 * m),
                   sts[m][0:K1, :], tile_position=(0, 0))
                mm(('A', 2 * m + 1), ps_ss[1][:], tab(2 * m + 1),
                   sts[m][64:64 + K1, :], tile_position=(64, 0))

            tensor.wait_ge(warm_sem, 1)
            for _ in range(7):
                warm_mm()
            tensor.wait_ge(c_sem, 16)
            tensor.wait_ge(st_sems[0], 16)
            a_pair(0)
            for j in range(BPC + LAG):
                if j % 2 == 1 and (j + 1) // 2 < BPC // 2:  # A pair (j+1)//2
                    m = (j + 1) // 2
                    tensor.wait_ge(st_sems[m], 16)
                    # both ps_s banks freed by the previous pair's exps
                    tensor.wait_ge(act_sem, act_tick[('exp', 2 * m - 1)])
                    a_pair(m)
                if j < BPC:                     # hs_j
                    tensor.wait_ge(act_sem, act_tick[('exp', j)])
                    if j == 0:
                        tensor.wait_ge(v_sem, 32)
                    if j in ZW_FREE:            # previous reader freed zw half
                        tensor.wait_ge(act_sem, act_tick[ZW_FREE[j]])
                    mm(('hs', j), zws[j % 2][:], seg2, ps[j % 6][:])
                if 0 <= j - LAG < BPC:          # m2_{j-LAG}
                    i = j - LAG
                    tensor.wait_ge(dve_sem, dve_tick[('mul', i)])
                    if i == 0:
                        tensor.wait_ge(v_sem, 32)
                    if i >= 2:                  # copy of row i-2 freed ps_o
                        tensor.wait_ge(dve_sem, dve_tick[('ocopy', i - 2)])
                    mm(('m2a', i), ps_os[i % 2][:, 0:S],
                       vproj[:, 0:128], ps[i % 6][:])
                    mm(('m2b', i), ps_os[i % 2][:, S:2 * S],
                       vproj[:, 128:256], ps[i % 6][:])


        @block.scalar
        def _(scalar):
            # preload the Exp/Ln PWP tables during the DMA flight window
            # (input is the preamble-initialized const-1.0 AP so CoreSim's
            # uninitialized-read check stays clean; scratch out into ot0)
            cap = nc.const_aps.aps[(f32, 1.0)]
            scalar.activation(lnz_sb[:4, 0:1], cap[0:4], Exp)
            scalar.activation(lnz_sb[:4, 1:2], cap[0:4], Ln)
            for key in act_order:
                kind, i = key
                if kind == 'exp':
                    scalar.wait_ge(pe_sem, pe_tick[('A', i)])
                    scalar.activation(ps[i % 6][:], ps_ss[i % 2][:],
                                      Exp).then_inc(act_sem, 1)
                elif kind == 'lnr':             # per-row ln, row i
                    sl = (i % 2) * S
                    scalar.wait_ge(pe_sem, pe_tick[('hs', i)])
                    if i >= 2:  # lnz-slice WAR: expnegr_{i-2} reads drained
                        scalar.wait_ge(act_sem, act_tick[('expnegr', i - 2)])
                    scalar.activation(lnz_sb[:, sl:sl + S],
                                      zws[i % 2][:],
                                      Ln).then_inc(act_sem, 1)
                elif kind == 'expnegr':     # per-row expneg, row i
                    sl = (i % 2) * S
                    if i >= 2:                  # mul_{i-2} freed r_sb slice
                        scalar.wait_ge(dve_sem, dve_tick[('mul', i - 2)])
                    scalar.wait_ge(act_sem, act_tick[('lnr', i)])
                    scalar.activation(r_sb[:, sl:sl + S],
                                      lnz_sb[:, sl:sl + S], Exp,
                                      scale=-1.0).then_inc(act_sem, 1)
                elif kind == 'acp':             # tail output copy, row i
                    scalar.wait_ge(pe_sem, pe_tick[('m2b', i)])
                    scalar.wait_ge(ot_sems[i - 3], 16)
                    scalar.copy(ots[i % 3][:],
                                ps_os[i % 2][:]).then_inc(act_sem, 1)
                else:  # acpB: tail output copy, row i's B half
                    scalar.wait_ge(pe_sem, pe_tick[('m2b', i)])
                    scalar.wait_ge(ot_sems[i - 3], 16)
                    scalar.copy(ots[i % 3][:, S:2 * S],
                                ps_os[i % 2][:, S:2 * S]).then_inc(act_sem, 1)

        @block.vector
        def _(vector):
            vector.memset(warm_sb[:], 0.0).then_inc(warm_sem, 1)
            for key in dve_order:
                kind, i = key
                if kind == 'mul':
                    vector.wait_ge(act_sem, act_tick[rkey(i)])
                    sl = (i % 2) * S
                    vector.tensor_mul(ps[i % 6][:HT, :], ps[i % 6][:HT, :],
                                      r_sb[:, sl:sl + S]).then_inc(dve_sem, 1)
                elif kind == 'ocopy':
                    vector.wait_ge(pe_sem, pe_tick[('m2b', i)])
                    if i >= 3:
                        vector.wait_ge(ot_sems[i - 3], 16)
                    vector.tensor_copy(ots[i % 3][:],
                                       ps_os[i % 2][:]).then_inc(dve_sem, 1)
                elif kind == 'ocopyA':
                    vector.wait_ge(pe_sem, pe_tick[('m2a', i)])
                    if i >= 3:
                        vector.wait_ge(ot_sems[i - 3], 16)
                    vector.tensor_copy(ots[i % 3][:, 0:S],
                                       ps_os[i % 2][:, 0:S]).then_inc(
                                           dve_sem, 1)


        @block.sync
        def _(sync):
            sync.dma_start(sts[0][:], stream_d[0]).then_inc(st_sems[0], 16)
            sync.dma_start(tabs_sb[:], tabs_d[:]).then_inc(c_sem, 16)
            sync.dma_start(vpseg_sb[0:49, :],
                           vpseg_d[0:49, :]).then_inc(v_sem, 16)
            sync.dma_start(vpseg_sb[49:TW, :],
                           vpseg_d[49:TW, :]).then_inc(v_sem, 16)
            sync.dma_start(sts[1][:], stream_d[1]).then_inc(st_sems[1], 16)
            for i in range(2, BPC // 2):
                sync.dma_start(sts[i][:], stream_d[i]).then_inc(st_sems[i], 16)
            for k in range(BPC - 1):
                if k == BPC - 2:
                    sync.wait_ge(act_sem, act_tick[('acp', k)])
                else:
                    sync.wait_ge(dve_sem, dve_tick[('ocopy', k)])
                dest = out_d[k, :, :].rearrange("(h p) s -> p h s", h=2)
                src = ots[k % 3][:, :].rearrange("p (h s) -> p h s", h=2)
                sync.dma_start(dest, src).then_inc(ot_sems[k], 16)
            k = BPC - 1
            sync.wait_ge(dve_sem, dve_tick[('ocopyA', k)])
            sync.dma_start(out_d[k, 0:128, :],
                           ots[k % 3][:, 0:S]).then_inc(ot_sems[k], 16)
            sync.wait_ge(act_sem, act_tick[('acpB', k)])
            sync.dma_start(out_d[k, 128:256, :],
                           ots[k % 3][:, S:2 * S]).then_inc(ot_sems[k], 16)
            for k in range(BPC - 1):
                sync.wait_ge(ot_sems[k], 16)
            sync.wait_ge(ot_sems[BPC - 1], 32)

    return nc


def _run(inputs, trace=False):
    import sys
    if "/opt/trn_rl_repo" not in sys.path:
        sys.path.insert(0, "/opt/trn_rl_repo")
    from concourse.bass_utils import run_bass_kernel_spmd

    tabs_cores, vpseg_bf, streams = _host_tables(**inputs)
    nc = _build_program()
    in_maps = [
        {"stream": streams[c], "tabs": tabs_cores[c], "vpseg": vpseg_bf}
        for c in range(NCORES)
    ]
    res = run_bass_kernel_spmd(nc, in_maps, core_ids=list(range(NCORES)),
                               trace=trace)
    out_full = np.empty((B, S, D), np.float32)
    for c in range(NCORES):
        oc = res.results[c]["out"]  # [BPC, D, S] bf16
        out_full[c * BPC:(c + 1) * BPC] = \
            oc.astype(np.float32).transpose(0, 2, 1)
    return out_full, res


def kernel(**inputs):
    trace = bool(int(os.environ.get("BASS_KERNEL_TRACE", "0")))
    out, _ = _run(inputs, trace=trace)
    return out


def kernel_profiled(**inputs):
    out, res = _run(inputs, trace=True)
    return out, res
